# revision 92
# baseline (speedup 1.0000x reference)
"""Trainium2 Bass kernel for multi-head attention graph scatter.

Computes, for each of 8 heads h (one NeuronCore per head):
    q_h = query @ w_q[:, h*32:(h+1)*32]          # [3000, 32]
    k_h = key_emb @ w_k[:, h*32:(h+1)*32]        # [4096, 32]
    attn_h = softmax(q_h @ k_h.T / sqrt(32))     # [3000, 4096]
    graphs[h, qt, :] = attn_h                    # [4096, 4096], rest zeros

kernel(**inputs) takes the full (unsharded) numpy inputs and returns the
full [8, 4096, 4096] float32 output.
"""

import math
import sys

import numpy as np

if "/opt/trn_rl_repo" not in sys.path:
    sys.path.insert(0, "/opt/trn_rl_repo")

N_HEAD = 8
D_K = 32
CONCEPT_NUM = 4096
MASK_NUM = 3000
INPUT_DIM = 256

P = 128  # SBUF partitions
NBLK = 512  # matmul moving-dim tile (one PSUM bank of f32)

_BUILD_CACHE = {}


def _build_module():
    """Build the per-core Bass module (identical on all 8 cores; inputs differ)."""
    import concourse.bacc as bacc
    import concourse.mybir as mybir
    import concourse.tile as tile
    from concourse.masks import make_identity

    f32 = mybir.dt.float32
    f32r = mybir.dt.float32r
    SCALE = 1.0 / math.sqrt(D_K)

    nc = bacc.Bacc("TRN2", target_bir_lowering=False, debug=False, num_devices=N_HEAD)

    query = nc.dram_tensor("query", [MASK_NUM, INPUT_DIM], f32, kind="ExternalInput")
    key_emb = nc.dram_tensor("key_emb", [CONCEPT_NUM, INPUT_DIM], f32, kind="ExternalInput")
    # both per-head weight slices, host-packed as [p, a, {q,k}, j] so one DMA
    # with 512B-contiguous rows loads them at full bus rate
    w_qk = nc.dram_tensor("w_qk", [P, 2, 2, D_K], f32, kind="ExternalInput")
    # w_k slice transposed ([d, a, p]): lets tile 0 fold the key projection
    # into its query vectors (A = w_k^T @ qT0) and compute scores straight
    # from the 256-dim transposed keys, skipping the kT projection chain
    w_t = nc.dram_tensor("w_t", [D_K, 2, P], f32, kind="ExternalInput")
    graphs = nc.dram_tensor("graphs", [CONCEPT_NUM, CONCEPT_NUM], f32, kind="ExternalOutput")

    # mask-dim tiling: 3000 = 23*128 + 56
    m_tiles = [P] * (MASK_NUM // P) + ([MASK_NUM % P] if MASK_NUM % P else [])
    n_mt = len(m_tiles)
    n_kc = CONCEPT_NUM // NBLK  # 8 concept chunks of 512
    q_chunks = [NBLK] * (MASK_NUM // NBLK) + ([MASK_NUM % NBLK] if MASK_NUM % NBLK else [])
    n_qc = len(q_chunks)  # 6 mask chunks (5x512 + 440)
    n_qt_full = MASK_NUM // P  # 23 full query row-tiles
    mrem = MASK_NUM - n_qt_full * P  # 56

    with tile.TileContext(nc) as tc:
        with (
            tc.tile_pool(name="const", bufs=1) as const_pool,
            tc.tile_pool(name="loads", bufs=6) as loads,
            tc.tile_pool(name="trans", bufs=1) as trans_pool,
            tc.tile_pool(name="proj", bufs=1) as proj_pool,
            tc.tile_pool(name="stats", bufs=4) as stats,
            tc.tile_pool(name="expp", bufs=4) as expp,
            tc.tile_pool(name="tpsum", bufs=3, space="PSUM") as tpsum,
            tc.tile_pool(name="ppsum", bufs=1, space="PSUM") as ppsum,
            tc.tile_pool(name="mpsum", bufs=2, space="PSUM") as mpsum,
        ):
            identity = const_pool.tile([P, P], f32)
            make_identity(nc, identity)

            # warm the PE clock (HAM) before the first real transposes arrive
            for _ in range(8):
                wtp = tpsum.tile([P, 4 * P], f32, tag="tp", name="wtp")
                nc.tensor.transpose(wtp[:, :P], identity[:], identity[:])

            # packed per-head w slices; cast to f32r for the projection matmuls
            wqk_f32 = const_pool.tile([P, 2, 2, D_K], f32)
            wqk_sb = const_pool.tile([P, 2, 2, D_K], f32r)
            wt_f32 = const_pool.tile([D_K, 2, P], f32)
            wt_sb = const_pool.tile([D_K, 2, P], f32r)
            a_sb = [
                [const_pool.tile([P, P], f32r, name=f"a_sb{ti}_{a}") for a in range(2)]
                for ti in range(2)
            ]

            def emit_w_load():
                nc.sync.dma_start(wqk_f32[:], w_qk.ap())
                nc.vector.tensor_copy(wqk_sb[:], wqk_f32[:])
                nc.sync.dma_start(wt_f32[:], w_t.ap())
                nc.vector.tensor_copy(wt_sb[:], wt_f32[:])

            def emit_a():
                """A_ti_a[p, m] = sum_d w_k[a*128+p, d] * qT0[d, ti*128+m] for
                tiles 0 and 1 (their first-chunk query rows both live in qT0).

                PSUM from tpsum, NOT ppsum: in the 1-buf ppsum rotation these
                allocs would push every kT projection ~1.3us later."""
                for ti in range(2):
                    for a in range(2):
                        pa = tpsum.tile([P, 4 * P], f32, tag="tp", name="pa")
                        nc.tensor.matmul(
                            pa[:, :P], wt_sb[:, a, :], qT[0][:, ti * P : (ti + 1) * P],
                            start=True, stop=True,
                        )
                        nc.vector.tensor_copy(a_sb[ti][a][:], pa[:, :P])

            def scores0(ps, c0, j, ti=0):
                """Scores for key chunk j of tile `ti` via the folded 256-dim
                form: reads keyT directly, no kT projection on the critical
                path."""
                nc.tensor.matmul(ps[:, c0 : c0 + NBLK], a_sb[ti][0][:], keyT[0][j][:], start=True, stop=False)
                nc.tensor.matmul(ps[:, c0 : c0 + NBLK], a_sb[ti][1][:], keyT[1][j][:], start=False, stop=True)

            # transposed input staging (f32r, rounded by the PSUM->SBUF copies)
            keyT = [
                [trans_pool.tile([P, NBLK], f32r, tag=f"keyT{a}_{j}", name=f"keyT{a}_{j}") for j in range(n_kc)]
                for a in range(2)
            ]
            queryT = [
                [trans_pool.tile([P, q_chunks[j]], f32r, tag=f"queryT{a}_{j}", name=f"queryT{a}_{j}") for j in range(n_qc)]
                for a in range(2)
            ]
            kT = [proj_pool.tile([D_K, NBLK], f32r, tag=f"kT_{j}", name=f"kT_{j}") for j in range(n_kc)]
            qT = [proj_pool.tile([D_K, q_chunks[j]], f32r, tag=f"qT_{j}", name=f"qT_{j}") for j in range(n_qc)]

            # ---------- helpers ----------
            def transpose_quad(srcs, dst, a, pool=None, eng=None):
                """PE-transpose up to 4 [rows<=128, 128] blocks into ONE PSUM
                tile, then one wide copy into dst. Copy engine: a=0 -> DVE,
                a=1 -> Act (strict split keeps each engine under the 1.46us
                per-chunk load cadence); `eng` overrides (the last key chunks
                put BOTH copies on DVE so Act's queue can't delay the tile-0
                exps that gate on them). Main-loop query quads pass the mpsum
                pool: its buffer rotation delays them behind the softmax
                stream, keeping PE/DVE/Act clear during the tile-0 critical
                window."""
                if pool is None:
                    tp = tpsum.tile([P, 4 * P], f32, tag="tp", name="tp")
                else:
                    tp = pool.tile([P, 4 * P], f32, tag="mps", name="qtp")
                w = 0
                for s in srcs:
                    r = s.shape[0]
                    nc.tensor.transpose(tp[:, w : w + r], s, identity[:r, :r])
                    w += r
                if (a if eng is None else eng) == 0:
                    nc.vector.tensor_copy(dst[:, :w], tp[:, :w])
                else:
                    nc.scalar.copy(dst[:, :w], tp[:, :w])

            def load_q(g):
                """Issue the DMA(s) for query row-tiles 4g..4g+3 (or the tail)."""
                qtile = loads.tile([P, 4, INPUT_DIM], f32, tag="ld", name="qload")
                t0 = g * 4
                t1 = min(t0 + 4, n_qt_full)
                eng = nc.sync if g == 0 else nc.gpsimd
                if t1 > t0:
                    src = query.ap()[t0 * P : t1 * P, :].rearrange("(t p) d -> p t d", p=P)
                    eng.dma_start(qtile[:, : t1 - t0, :], src)
                if g == 5:
                    nc.gpsimd.dma_start(qtile[:mrem, 3, :], query.ap()[n_qt_full * P :, :])
                return qtile

            def process_q(g, qtile):
                """Transpose a loaded query group into queryT (deferred from
                load time so startup copy engines stay on the key chunks)."""
                t0 = g * 4
                n_full = min(t0 + 4, n_qt_full) - t0
                for a in range(2):
                    srcs = [qtile[:, t, a * P : (a + 1) * P] for t in range(n_full)]
                    if g == 5:
                        srcs.append(qtile[:mrem, 3, a * P : (a + 1) * P])
                    transpose_quad(srcs, queryT[a][g], a, pool=None if g == 0 else mpsum)

            def project(dst, srcT, s_idx, width, pool=None, eng=0):
                if pool is None:
                    ps = ppsum.tile([D_K, NBLK], f32, tag="pps", name="pps")
                else:
                    ps = pool.tile([P, 2 * NBLK], f32, tag="mps", name="kproj")
                nc.tensor.matmul(ps[:D_K, :width], wqk_sb[:, 0, s_idx, :], srcT[0][:], start=True, stop=False)
                nc.tensor.matmul(ps[:D_K, :width], wqk_sb[:, 1, s_idx, :], srcT[1][:], start=False, stop=True)
                if eng == 0:
                    nc.vector.tensor_copy(dst[:], ps[:D_K, :width])
                else:
                    nc.scalar.copy(dst[:], ps[:D_K, :width])

            def scores_chunk(i, ps, j2_off, j):
                """One [mt, 512] scores matmul for m-tile i into psum slice j2_off."""
                mt = m_tiles[i]
                cj = i // 4
                c0 = i * P - cj * NBLK
                nc.tensor.matmul(
                    ps[:mt, j2_off * NBLK : (j2_off + 1) * NBLK],
                    qT[cj][:, c0 : c0 + mt],
                    kT[j][:],
                    start=True,
                    stop=True,
                )

            def exp_chunk(i, ps, exp_dst, sums, h4):
                mt = m_tiles[i]
                nc.scalar.activation(
                    exp_dst[:mt, h4 * 2 * NBLK : (h4 + 1) * 2 * NBLK],
                    ps[:mt, :],
                    mybir.ActivationFunctionType.Exp,
                    scale=SCALE,
                    accum_out=sums[:mt, h4 : h4 + 1],
                )

            def normalize_write(i, exp_dst, sums, widths):
                """Reduce+reciprocal, then normalize and write the row tile in
                column slices of the given widths so the first HBM write
                enters the DMA queue before the whole row is normalized
                (latency matters for the first few tiles; later tiles have
                slack)."""
                mt = m_tiles[i]
                tot = stats.tile([P, 1], f32, tag="tot", name="tot")
                rec = stats.tile([P, 1], f32, tag="rec", name="rec")
                nc.vector.tensor_reduce(
                    tot[:mt], sums[:mt, :], axis=mybir.AxisListType.X, op=mybir.AluOpType.add
                )
                nc.vector.reciprocal(rec[:mt], tot[:mt])
                eng = nc.sync if i % 2 == 0 else nc.gpsimd
                c0 = 0
                for w in widths:
                    nc.vector.tensor_scalar_mul(
                        exp_dst[:mt, c0 : c0 + w], exp_dst[:mt, c0 : c0 + w], rec[:mt]
                    )
                    eng.dma_start(
                        graphs.ap()[i * P : i * P + mt, c0 : c0 + w],
                        exp_dst[:mt, c0 : c0 + w],
                    )
                    c0 += w
                assert c0 == CONCEPT_NUM

            def softmax_tile(i, exp_dst):
                """Full scores+exp for m-tile i into exp_dst [P, C]."""
                sums = stats.tile([P, 4], f32, tag="sums", name="sums")
                for h4 in range(4):
                    ps = mpsum.tile([P, 2 * NBLK], f32, tag="mps", name="mps")
                    if i == 1 and h4 == 3:
                        # tile 1 takes the A-form for chunks {6,7} so nothing
                        # it needs waits on the deferred kT_7 projection
                        scores0(ps, 0, 6, ti=1)
                        scores0(ps, NBLK, 7, ti=1)
                    else:
                        for j2 in range(2):
                            scores_chunk(i, ps, j2, h4 * 2 + j2)
                    exp_chunk(i, ps, exp_dst, sums, h4)
                return sums

            # ---------- startup: query chunk 0, then key side with tile-0
            # softmax interleaved so the first output DMA starts ASAP ----------
            q0_tile = load_q(0)
            process_q(0, q0_tile)

            key_r = key_emb.ap().rearrange("(t p) d -> p t d", p=P)  # [128, 32, 256]
            exp0 = expp.tile([P, CONCEPT_NUM], f32, tag="exp", name="exp0")
            # 5 partial sums: chunk pairs {0,1},{2,3},{4,5} then singles {6},{7}
            # (the last exp on the critical path covers only the last-loaded
            # key chunk, halving its Act time)
            sums0 = stats.tile([P, 5], f32, tag="sums", name="sums0")
            ps0 = None
            for j in range(n_kc):  # 8 key groups of 4 row-tiles (0.5 MB loads)
                ktile = loads.tile([P, 4, INPUT_DIM], f32, tag="ld", name="kload")
                nc.sync.dma_start(ktile[:], key_r[:, j * 4 : (j + 1) * 4, :])
                for a in range(2):
                    transpose_quad(
                        [ktile[:, t, a * P : (a + 1) * P] for t in range(4)],
                        keyT[a][j],
                        a,
                        eng=0 if j >= 6 else None,
                    )
                if j == 0:
                    emit_w_load()
                    project(qT[0], [queryT[0][0], queryT[1][0]], 0, q_chunks[0])
                    emit_a()
                if j < 7:
                    project(kT[j], [keyT[0][j], keyT[1][j]], 1, NBLK)
                # tile-0 scores: chunks 0-5 via kT (off critical path), the
                # last two via the folded A-form reading keyT directly, so
                # nothing on tile-0's tail waits for a kT projection
                if j < 6:
                    if j % 2 == 0:
                        ps0 = mpsum.tile([P, 2 * NBLK], f32, tag="mps", name="mps")
                    scores_chunk(0, ps0, j % 2, j)
                    if j % 2 == 1:
                        exp_chunk(0, ps0, exp0, sums0, j // 2)
                else:  # singles for the last two chunks via the A-form, one
                    # shared alloc (separate banks): each exp gates only on
                    # its own chunk's transpose copies
                    if j == 6:
                        ps0 = mpsum.tile([P, 2 * NBLK], f32, tag="mps", name="mps1")
                    scores0(ps0, (j - 6) * NBLK, j)
                    nc.scalar.activation(
                        exp0[:, j * NBLK : (j + 1) * NBLK],
                        ps0[:, (j - 6) * NBLK : (j - 5) * NBLK],
                        mybir.ActivationFunctionType.Exp,
                        scale=SCALE,
                        accum_out=sums0[:, j - 3 : j - 2],
                    )

            # issue the remaining query loads now: they pack the DMA device
            # right behind the key loads while tile-0's softmax tail drains
            q_tiles = {0: q0_tile}
            for g in range(1, n_qc):
                q_tiles[g] = load_q(g)

            # tile 0 write in eighths: the first 512-col slice enters the DMA
            # queue ~1.5us earlier than a half-row would
            normalize_write(0, exp0, sums0, [512] * 8)

            # ---------- main loop; query chunk g transposed+projected just
            # before the first tile that needs it (i = 4g) so PE doesn't pull
            # the transposes forward into the tile-0 critical window ----------
            done_qc = 1
            for i in range(1, n_mt):
                if i == 4 * done_qc - 2 and done_qc < n_qc:
                    process_q(done_qc, q_tiles[done_qc])
                    project(qT[done_qc], [queryT[0][done_qc], queryT[1][done_qc]], 0, q_chunks[done_qc])
                    done_qc += 1
                if i == 2:
                    # deferred kT_7 projection (tile 1 uses the A-form for
                    # chunks {6,7}; tile 2 is the first reader). Behind qT_1
                    # in the 1-buf ppsum rotation its DVE copy becomes ready
                    # only after the tile-0 reduce has claimed the engine
                    project(kT[7], [keyT[0][7], keyT[1][7]], 1, NBLK)
                exp_t = expp.tile([P, CONCEPT_NUM], f32, tag="exp", name="exp_t")
                sums = softmax_tile(i, exp_t)
                normalize_write(i, exp_t, sums, [1024] * 4 if i <= 10 else [4096])
            while done_qc < n_qc:  # safety (should not trigger)
                process_q(done_qc, q_tiles[done_qc])
                project(qT[done_qc], [queryT[0][done_qc], queryT[1][done_qc]], 0, q_chunks[done_qc])
                done_qc += 1

    nc.compile()
    return nc


def _get_module():
    if "nc" not in _BUILD_CACHE:
        _BUILD_CACHE["nc"] = _build_module()
    return _BUILD_CACHE["nc"]


def _pack_w(w_q, w_k, h):
    """Per-head weight slices packed as [p, a, {q,k}, j] with 512B rows."""
    wq = w_q[:, h * D_K : (h + 1) * D_K].reshape(2, P, D_K)
    wk = w_k[:, h * D_K : (h + 1) * D_K].reshape(2, P, D_K)
    out = np.empty((P, 2, 2, D_K), np.float32)
    out[:, :, 0, :] = wq.transpose(1, 0, 2)
    out[:, :, 1, :] = wk.transpose(1, 0, 2)
    return np.ascontiguousarray(out)


def _pack_wt(w_k, h):
    """Transposed per-head w_k slice, [d, a, p] layout."""
    wk = w_k[:, h * D_K : (h + 1) * D_K].reshape(2, P, D_K)  # [a, p, d]
    return np.ascontiguousarray(wk.transpose(2, 0, 1))


def kernel(qt, query, key_emb, w_q, w_k):
    from concourse.bass_utils import run_bass_kernel_spmd

    qt = np.asarray(qt)
    query = np.ascontiguousarray(np.asarray(query, dtype=np.float32))
    key_emb = np.ascontiguousarray(np.asarray(key_emb, dtype=np.float32))
    w_q = np.asarray(w_q, dtype=np.float32)
    w_k = np.asarray(w_k, dtype=np.float32)

    nc = _get_module()
    in_maps = []
    for h in range(N_HEAD):
        in_maps.append(
            {
                "query": query,
                "key_emb": key_emb,
                "w_qk": _pack_w(w_q, w_k, h),
                "w_t": _pack_wt(w_k, h),
            }
        )
    res = run_bass_kernel_spmd(nc, in_maps, core_ids=list(range(N_HEAD)))
    out = np.stack([res.results[h]["graphs"] for h in range(N_HEAD)], axis=0)

    # Device assumes qt == arange(3000) (rows land at graph rows 0..2999,
    # remaining rows stay zero). Remap on host for any other qt.
    if not np.array_equal(qt, np.arange(MASK_NUM)):
        full = np.zeros((N_HEAD, CONCEPT_NUM, CONCEPT_NUM), dtype=np.float32)
        full[:, qt.astype(np.int64), :] = out[:, :MASK_NUM, :]
        out = full
    return out
